# revision 1
# baseline (speedup 1.0000x reference)
"""GRU decoder kernel for Trainium2, 8 NeuronCores.

Sharding: recurrence replicated on all cores (it is serial); the vocab
projection (dominant cost) is column-sharded 8 ways (4000 vocab rows per
core). Zero collectives; every core runs the same program on different
W_out/b_out shards. Host only reshapes/casts/slices inputs and
concatenates the output shards.

Pipeline per core:
  A: gc = context @ W_ic.T (+ biases)          [tiny]
  B: gather emb rows by token, PE-transpose, gi = words @ W_iw.T + gc
     -> gi scratch in DRAM (bf16)
  C: 64 sequential GRU steps. Per step: gh = h @ W_hh.T as 3 col-tiled
     psum pairs (two concurrent tile_position matmuls), gi folded in via
     identity-matmul accumulation; gates on ACT/DVE; h transposed back
     with 4 PE transposes. The output projection for finished steps is
     interleaved into the PE stream to fill the gate-latency gaps.
"""
import sys
sys.path.insert(0, '/opt/trn_rl_repo')
import numpy as np
import ml_dtypes

import concourse.bass as bass
import concourse.bacc as bacc
import concourse.mybir as mybir
import concourse.tile as tile
from concourse.bass import IndirectOffsetOnAxis
from concourse.bass_utils import run_bass_kernel_spmd
from concourse.masks import make_identity

B, T, V, DE, DD, DC = 64, 64, 32000, 512, 1024, 512
NCORES = 8
VS = V // NCORES        # vocab shard per core
BT = B * T              # 4096 (row order: bt = t*B + b)
MT = BT // 128          # 32 m-tiles, m-tile m = steps 2m, 2m+1
GD = 3 * DD             # 3072 gate dim
KD = DD // 128          # 8 hidden k-chunks
KE = DE // 128          # 4 emb/ctx k-chunks
NPROJ = 8               # proj n-chunks per m-tile
PN = VS // NPROJ        # 500
BF = mybir.dt.bfloat16
F32 = mybir.dt.float32
AF = mybir.ActivationFunctionType
OP = mybir.AluOpType

_cache = {}


def _build(with_bhn=True):
    key = ("nc", with_bhn)
    if key in _cache:
        return _cache[key]
    nc = bacc.Bacc("TRN2", target_bir_lowering=False, debug=False,
                   num_devices=NCORES)
    dt = nc.dram_tensor
    emb16 = dt("emb16", [V, DE], BF, kind="ExternalInput").ap()
    tok = dt("tok", [BT, 1], mybir.dt.int32, kind="ExternalInput").ap()
    WiwT = dt("WiwT", [128, KE, GD], BF, kind="ExternalInput").ap()
    WicT = dt("WicT", [128, KE, GD], BF, kind="ExternalInput").ap()
    ctxT = dt("ctxT", [128, KE, B], BF, kind="ExternalInput").ap()
    Whh = dt("Whh", [128, KD, GD], BF, kind="ExternalInput").ap()
    Wout = dt("Wout", [128, KD, VS], BF, kind="ExternalInput").ap()
    biasgi2 = dt("biasgi2", [128, GD], F32, kind="ExternalInput").ap()
    bhn2 = dt("bhn2", [B, DD], BF, kind="ExternalInput").ap()
    bout2 = dt("bout2", [128, VS], BF, kind="ExternalInput").ap()
    initT = dt("initT", [128, KD, 2, B], BF, kind="ExternalInput").ap()
    initp = dt("initp", [128, DD // 2], F32, kind="ExternalInput").ap()
    o = dt("o", [MT, 128, VS], F32, kind="ExternalOutput").ap()

    with tile.TileContext(nc) as tc:
        with tc.tile_pool(name="dram", bufs=1, space="DRAM") as dpool, \
             tc.tile_pool(name="const", bufs=1) as cpool:
            gi_dram = dpool.tile([MT, 128, GD], BF)

            ident = cpool.tile([128, 128], BF)
            make_identity(nc, ident[:])
            i64 = cpool.tile([64, 64], BF)
            make_identity(nc, i64[:])
            c_whh = cpool.tile([128, KD, GD], BF)
            c_wout = cpool.tile([128, KD, VS], BF)
            c_bout = cpool.tile([128, VS], BF)
            c_bhn = cpool.tile([B, DD], BF)

            # ================= phase A + B: gi = words@WiwT + ctx@WicT + biases
            with tc.tile_pool(name="bconst", bufs=1) as bc, \
                 tc.tile_pool(name="bwork", bufs=3) as bw, \
                 tc.tile_pool(name="bwt", bufs=2) as bwt, \
                 tc.tile_pool(name="bgi", bufs=2) as bgi, \
                 tc.tile_pool(name="bps", bufs=4, space="PSUM") as bps, \
                 tc.tile_pool(name="btps", bufs=2, space="PSUM") as btps:
                c_wiw = bc.tile([128, KE, GD], BF)
                nc.sync.dma_start(c_wiw[:], WiwT)
                c_wic = bc.tile([128, KE, GD], BF)
                nc.sync.dma_start(c_wic[:], WicT)
                c_ctx = bc.tile([128, KE, B], BF)
                nc.sync.dma_start(c_ctx[:], ctxT)
                c_bgi = bc.tile([128, GD], F32)
                nc.sync.dma_start(c_bgi[:], biasgi2)
                gc2 = bc.tile([128, GD], BF)

                # gc chunks: [64, 512] psum; add biases; duplicate to both halves
                for c in range(6):
                    ps = bps.tile([64, 512], F32, tag="gwps")
                    for k in range(KE):
                        nc.tensor.matmul(ps[:], c_ctx[:, k, :],
                                         c_wic[:, k, c * 512:(c + 1) * 512],
                                         start=(k == 0), stop=(k == KE - 1))
                    sl = slice(c * 512, (c + 1) * 512)
                    nc.vector.tensor_tensor(gc2[0:64, sl], ps[:],
                                            c_bgi[0:64, sl], op=OP.add)
                    nc.vector.tensor_tensor(gc2[64:128, sl], ps[:],
                                            c_bgi[64:128, sl], op=OP.add)

                for m in range(MT):
                    tokt = bw.tile([128, 1], mybir.dt.int32, tag="tokt")
                    nc.sync.dma_start(tokt[:], tok[m * 128:(m + 1) * 128, :])
                    wrow = bw.tile([128, DE], BF, tag="wrow")
                    nc.gpsimd.indirect_dma_start(
                        out=wrow[:], out_offset=None, in_=emb16[:, :],
                        in_offset=IndirectOffsetOnAxis(ap=tokt[:, :1], axis=0))
                    wT = bwt.tile([128, KE, 128], BF, tag="wT")
                    for b in range(KE):
                        tp = btps.tile([128, 128], BF, tag="tpsB")
                        nc.tensor.transpose(tp[:], wrow[:, b * 128:(b + 1) * 128],
                                            ident[:])
                        nc.scalar.copy(wT[:, b, :], tp[:])
                    gist = bgi.tile([128, GD], BF, tag="gist")
                    for c in range(6):
                        ps = bps.tile([128, 512], F32, tag="gwps")
                        for k in range(KE):
                            nc.tensor.matmul(ps[:], wT[:, k, :],
                                             c_wiw[:, k, c * 512:(c + 1) * 512],
                                             start=(k == 0), stop=(k == KE - 1))
                        sl = slice(c * 512, (c + 1) * 512)
                        nc.vector.tensor_tensor(gist[:, sl], ps[:], gc2[:, sl],
                                                op=OP.add)
                    nc.sync.dma_start(gi_dram[m], gist[:])

            # big phase-C constants stream in while phase B computes
            nc.sync.dma_start(c_whh[:], Whh)
            nc.sync.dma_start(c_wout[:], Wout)
            nc.sync.dma_start(c_bout[:], bout2)
            nc.sync.dma_start(c_bhn[:], bhn2)

            # ================= phase C: recurrence + interleaved projection
            with tc.tile_pool(name="st2p", bufs=3) as st2p, \
                 tc.tile_pool(name="gstp", bufs=3) as gstp, \
                 tc.tile_pool(name="hp", bufs=2) as hp, \
                 tc.tile_pool(name="gates", bufs=2) as gp, \
                 tc.tile_pool(name="stgp", bufs=2) as stgp, \
                 tc.tile_pool(name="recps", bufs=2, space="PSUM") as recps, \
                 tc.tile_pool(name="ginps", bufs=1, space="PSUM") as ginps, \
                 tc.tile_pool(name="tps", bufs=1, space="PSUM") as tpsp, \
                 tc.tile_pool(name="projps", bufs=4, space="PSUM") as pps:
                st2_init = st2p.tile([128, KD, 2, B], BF, tag="st2")
                nc.sync.dma_start(st2_init[:], initT)
                h_prev = hp.tile([128, DD // 2], F32, tag="h")
                nc.sync.dma_start(h_prev[:], initp)
                st2_tiles = {-1: st2_init}

                def proj(tp_idx, grp):
                    stv = st2_tiles[tp_idx]
                    pt = [pps.tile([128, PN], F32, tag="projps",
                                   name=f"projps_{tp_idx}_{grp}_{i}")
                          for i in range(4)]
                    for k in range(KD):
                        for i in range(4):
                            c = grp * 4 + i
                            nc.tensor.matmul(
                                pt[i][:], stv[:, k, :, :],
                                c_wout[:, k, c * PN:(c + 1) * PN],
                                start=(k == 0), stop=(k == KD - 1))
                    stg = stgp.tile([128, 4, PN], F32, tag="stg")
                    for i in range(4):
                        c = grp * 4 + i
                        nc.vector.tensor_tensor(
                            stg[:, i, :], pt[i][:],
                            c_bout[:, c * PN:(c + 1) * PN], op=OP.add)
                    nc.sync.dma_start(
                        o[tp_idx, :, grp * 4 * PN:(grp + 1) * 4 * PN],
                        stg[:])

                for t in range(T):
                    m2, half = divmod(t, 2)
                    gstep = gstp.tile([64, GD], BF, tag="gstep")
                    nc.sync.dma_start(
                        gstep[:], gi_dram[m2, half * 64:half * 64 + 64, :])
                    prev = st2_tiles[(t - 1) // 2]
                    ph = (t - 1) % 2

                    def hh_pair(psum_t, c0, gi_c0, with_gi):
                        # chunks c0, c0+1 of gh into psum halves via
                        # concurrent col-tiled matmuls
                        last = (not with_gi) and (not with_bhn)
                        for k in range(KD):
                            nc.tensor.matmul(
                                psum_t[0:64, :], prev[:, k, ph, :],
                                c_whh[:, k, c0 * 512:(c0 + 1) * 512],
                                start=(k == 0), stop=(last and k == KD - 1),
                                tile_position=(0, 0))
                            nc.tensor.matmul(
                                psum_t[64:128, :], prev[:, k, ph, :],
                                c_whh[:, k, (c0 + 1) * 512:(c0 + 2) * 512],
                                start=(k == 0), stop=(last and k == KD - 1),
                                tile_position=(0, 64))
                        if with_gi:
                            nc.tensor.matmul(
                                psum_t[0:64, :], i64[:],
                                gstep[:, gi_c0 * 512:(gi_c0 + 1) * 512],
                                start=False, stop=True, tile_position=(0, 0))
                            nc.tensor.matmul(
                                psum_t[64:128, :], i64[:],
                                gstep[:, (gi_c0 + 1) * 512:(gi_c0 + 2) * 512],
                                start=False, stop=True, tile_position=(0, 64))
                        elif with_bhn:
                            # n-pair: add b_hh_n (inside the r-multiplied term)
                            nc.tensor.matmul(
                                psum_t[0:64, :], i64[:], c_bhn[:, 0:512],
                                start=False, stop=True, tile_position=(0, 0))
                            nc.tensor.matmul(
                                psum_t[64:128, :], i64[:], c_bhn[:, 512:1024],
                                start=False, stop=True, tile_position=(0, 64))

                    # r pair (gate chunks 0,1), n pair (4,5), z pair (2,3)
                    ps_r = recps.tile([128, 512], F32, tag="recps")
                    hh_pair(ps_r, 0, 0, True)
                    ps_n = recps.tile([128, 512], F32, tag="recps")
                    hh_pair(ps_n, 4, 0, False)
                    # gi_n into its own psum pair (identity matmuls)
                    ps_gin = ginps.tile([128, 512], F32, tag="ginps")
                    nc.tensor.matmul(ps_gin[0:64, :], i64[:],
                                     gstep[:, 2048:2560],
                                     start=True, stop=True,
                                     tile_position=(0, 0))
                    nc.tensor.matmul(ps_gin[64:128, :], i64[:],
                                     gstep[:, 2560:3072],
                                     start=True, stop=True,
                                     tile_position=(0, 64))
                    ps_z = recps.tile([128, 512], F32, tag="recps")
                    hh_pair(ps_z, 2, 2, True)

                    # projection for m-tile t//2-1 fills the gate latency gap
                    if t >= 2:
                        proj(m2 - 1, half)

                    # gates
                    r = gp.tile([128, 512], F32, tag="r")
                    nc.scalar.activation(r[:], ps_r[:], AF.Sigmoid)
                    tmp = gp.tile([128, 512], F32, tag="tmp")
                    nc.vector.tensor_tensor(tmp[:], r[:], ps_n[:], op=OP.mult)
                    tmp2 = gp.tile([128, 512], F32, tag="tmp2")
                    nc.vector.tensor_tensor(tmp2[:], tmp[:], ps_gin[:],
                                            op=OP.add)
                    n = gp.tile([128, 512], F32, tag="n")
                    nc.scalar.activation(n[:], tmp2[:], AF.Tanh)
                    z = gp.tile([128, 512], F32, tag="z")
                    nc.scalar.activation(z[:], ps_z[:], AF.Sigmoid)
                    u = gp.tile([128, 512], F32, tag="u")
                    nc.vector.tensor_tensor(u[:], z[:], h_prev[:], op=OP.mult)
                    w1z = gp.tile([128, 512], F32, tag="w1z")
                    nc.vector.tensor_scalar(w1z[:], z[:], -1.0, 1.0,
                                            OP.mult, OP.add)
                    v = gp.tile([128, 512], F32, tag="v")
                    nc.vector.tensor_tensor(v[:], w1z[:], n[:], op=OP.mult)
                    h_new = hp.tile([128, DD // 2], F32, tag="h")
                    nc.vector.tensor_tensor(h_new[:], u[:], v[:], op=OP.add)
                    h16 = gp.tile([128, DD // 2], BF, tag="h16")
                    nc.scalar.copy(h16[:], h_new[:])

                    # transpose h16 pair -> st2 slot for this step
                    if half == 0:
                        st2_cur = st2p.tile([128, KD, 2, B], BF, tag="st2")
                        st2_tiles[m2] = st2_cur
                    else:
                        st2_cur = st2_tiles[m2]
                    for b in range(4):
                        tp = tpsp.tile([128, 128], BF, tag="tps")
                        nc.tensor.transpose(
                            tp[:], h16[:, b * 128:(b + 1) * 128], ident[:])
                        src = tp[:].rearrange("p (u b) -> p u b", u=2)
                        if b % 2 == 0:
                            nc.scalar.copy(st2_cur[:, b::4, half, :], src)
                        else:
                            nc.vector.tensor_copy(st2_cur[:, b::4, half, :],
                                                  src)
                    h_prev = h_new

                # tail: last two m-tiles
                proj(MT - 2, 1)
                proj(MT - 1, 0)
                proj(MT - 1, 1)

    nc.compile()
    _cache[key] = nc
    return nc


def _prep_inputs(context, labels, emb, W_ih, b_ih, W_hh, b_hh, init,
                 W_out, b_out, bos_idx):
    bf = ml_dtypes.bfloat16
    labels = np.asarray(labels)
    tokens = np.concatenate(
        [np.full((B, 1), int(bos_idx), labels.dtype), labels[:, :-1]], axis=1)
    tok = np.ascontiguousarray(tokens.T.reshape(BT, 1)).astype(np.int32)

    emb16 = np.asarray(emb, np.float32).astype(bf)
    W_ih = np.asarray(W_ih, np.float32)
    WiwT = np.ascontiguousarray(
        W_ih[:, :DE].T.reshape(KE, 128, GD).transpose(1, 0, 2)).astype(bf)
    WicT = np.ascontiguousarray(
        W_ih[:, DE:].T.reshape(KE, 128, GD).transpose(1, 0, 2)).astype(bf)
    ctxT = np.ascontiguousarray(
        np.asarray(context, np.float32).T.reshape(KE, 128, B)
        .transpose(1, 0, 2)).astype(bf)
    WhhT = np.ascontiguousarray(
        np.asarray(W_hh, np.float32).T.reshape(KD, 128, GD)
        .transpose(1, 0, 2)).astype(bf)

    b_ih = np.asarray(b_ih, np.float32)
    b_hh = np.asarray(b_hh, np.float32)
    bias_gi = b_ih.copy()
    bias_gi[:2 * DD] += b_hh[:2 * DD]          # b_hh for r,z folded into gi
    biasgi2 = np.ascontiguousarray(
        np.broadcast_to(bias_gi[None, :], (128, GD))).astype(np.float32)
    bhn2 = np.ascontiguousarray(
        np.broadcast_to(b_hh[2 * DD:][None, :], (B, DD))).astype(bf)

    init = np.asarray(init, np.float32)
    h0 = init[0]
    initp = np.empty((128, DD // 2), np.float32)
    initp[0:64] = np.broadcast_to(h0[:DD // 2], (64, DD // 2))
    initp[64:128] = np.broadcast_to(h0[DD // 2:], (64, DD // 2))
    initT = np.zeros((128, KD, 2, B), np.float32)
    for k in range(KD):
        initT[:, k, 1, :] = np.broadcast_to(
            h0[k * 128:(k + 1) * 128][:, None], (128, B))
    initT = initT.astype(bf)

    W_out = np.asarray(W_out, np.float32)
    b_out = np.asarray(b_out, np.float32)
    in_maps = []
    for c in range(NCORES):
        ws = W_out[c * VS:(c + 1) * VS, :]
        WoutT = np.ascontiguousarray(
            ws.T.reshape(KD, 128, VS).transpose(1, 0, 2)).astype(bf)
        bout2 = np.ascontiguousarray(
            np.broadcast_to(b_out[c * VS:(c + 1) * VS][None, :],
                            (128, VS))).astype(bf)
        in_maps.append({
            "emb16": emb16, "tok": tok, "WiwT": WiwT, "WicT": WicT,
            "ctxT": ctxT, "Whh": WhhT, "Wout": WoutT, "biasgi2": biasgi2,
            "bhn2": bhn2, "bout2": bout2, "initT": initT, "initp": initp,
        })
    return in_maps


def kernel(**inputs) -> np.ndarray:
    b_hh = np.asarray(inputs["b_hh"], np.float32)
    nc = _build(with_bhn=bool(np.any(b_hh[2 * DD:])))
    in_maps = _prep_inputs(**inputs)
    res = run_bass_kernel_spmd(nc, in_maps, core_ids=list(range(NCORES)))
    shards = []
    for c in range(NCORES):
        oc = res.results[c]["o"].reshape(BT, VS)       # rows bt = t*B + b
        shards.append(oc.reshape(T, B, VS).transpose(1, 0, 2))
    return np.concatenate(shards, axis=2).astype(np.float32)

